# revision 15
# baseline (speedup 1.0000x reference)
"""Trainium2 Bass kernel for an XNOR-Net BasicBlock (dense_cnn).

Computes, for x [64,256,56,56] (NCHW):
    h = xnor_conv3x3(x, w1) -> bn1 -> hardtanh -> xnor_conv3x3 -> bn2
    out = relu(h + x)

where xnor_conv binarizes activations with sign() and weights with
sign()*mean(|w|) (per output channel).

Strategy (v4, fp8 DoubleRow):
  - Data-parallel over batch: 8 images per NeuronCore x 8 cores.
  - Binarized activations (+-1) are exact in fp8e4; conv = 9 shifted
    matmuls per 3x3 tap with fp32 PSUM accumulation (exact integers).
  - perf_mode=DoubleRow contracts K=256 (both 128-channel blocks) per
    matmul: lhsT [128,2,128], rhs [128,2,448]. DoubleRow requires a 3D
    rhs AP with contiguous N, so sign planes are stored 3x, one copy per
    kj column shift, with row stride 56 (58 rows x 56 cols, borders 0).
    Window for tap (ki,kj), out-row-chunk r0 is then the contiguous run
    plane[kj][:, :, (r0+ki)*W : +N].
  - Chunks are processed in pairs sharing one 2-bank PSUM tile [128,896]
    (each matmul still targets a single bank), halving evacuation ops.
  - Epilogue fusions: conv1 evac = Sign(a1*psum + c1) on ScalarE writing
    the kj=1 plane (DVE makes the kj=0/2 shifted copies); conv2 evac =
    DVE (psum*a2)+x then ScalarE Relu(. + c2). All per-channel constants
    (alpha, bn scale/bias) are folded on the host. hardtanh is a no-op
    for the final output because conv2 only consumes sign(h).

Layouts (per core):
  x DRAM     [8, 2, 128, 3136]   (img, c_blk, c_in_blk, h*w) fp32
  w DRAM     [2, 128, 9, 2, 128] (co_blk, ci, tap, ci_blk, co) fp8 sign
  cn DRAM    [2, 128, 4]         (co_blk, co, {a1,c1,a2,c2}) fp32
  out DRAM   [8, 2, 128, 3136]   (img, co_blk, co, h*w) fp32
"""

import os
import numpy as np

N, C, H, W = 64, 256, 56, 56
EPS = 1e-5
N_CORES = 8
IMG_PER_CORE = N // N_CORES
A = 2                     # channel blocks of 128
ROWS = H + 2              # padded rows in a plane
PLANE = ROWS * W          # 3248 (multiple of 16 for DoubleRow dim1 step)
RCH = 8                   # output rows per PSUM chunk
CHUNK = RCH * W           # 448 fp32 <= 512 (one PSUM bank)
HW = H * W
NCHUNK = HW // CHUNK      # 7
GROUPS = [(0, 1), (2, 3), (4, 5), (6,)]   # chunk pairs -> one PSUM tile
GROUPS_I0 = [(0,), (1, 2), (3, 4), (5, 6)]  # img 0: tiny first group (fast start)
# img 0 conv1 (b, group-idx) order: interleave the two output halves so PE
# consumption tracks the banded x/w DMA arrival order
CONV1_I0_ORDER = [(0, 0), (0, 1), (1, 0), (0, 2), (1, 1), (0, 3), (1, 2), (1, 3)]
BANDS_I0 = ((0, 9), (9, 17), (17, 25), (25, 41), (41, 56))  # x row-bands
# kj=1 taps first (plane-prep overlap); ki=1 first within each kj group so
# the start=True matmul is always full-width (ki=0/2 taps are trimmed at the
# top/bottom output-row chunks where they'd only read the zero pad row)
TAPS = [4, 1, 7, 3, 0, 6, 5, 2, 8]

_CACHE = {}
LAST_RESULT = None


def _build_program(n_img):
    import concourse.bacc as bacc
    import concourse.mybir as mybir
    import concourse.tile as tile

    dt = mybir.dt
    AF = mybir.ActivationFunctionType
    OP = mybir.AluOpType
    DR = mybir.MatmulPerfMode.DoubleRow

    nc = bacc.Bacc("TRN2", target_bir_lowering=False, debug=False)

    x_d = nc.dram_tensor("x", [n_img, A, 128, HW], dt.float32, kind="ExternalInput")
    w1_d = nc.dram_tensor("w1t", [A, 128, 9, A, 128], dt.float8e4, kind="ExternalInput")
    w2_d = nc.dram_tensor("w2t", [A, 128, 9, A, 128], dt.float8e4, kind="ExternalInput")
    cn_d = nc.dram_tensor("cn", [A, 128, 4], dt.float32, kind="ExternalInput")
    out_d = nc.dram_tensor("out", [n_img, A, 128, HW], dt.float32, kind="ExternalOutput")

    with tile.TileContext(nc) as tc:
        with (
            tc.tile_pool(name="consts", bufs=1) as consts,
            tc.tile_pool(name="planes", bufs=1) as planes,
            tc.tile_pool(name="xin", bufs=2) as xin,
            tc.tile_pool(name="outp", bufs=1) as outp,
            tc.tile_pool(name="evac", bufs=3) as evac,
            # far (right) side of SBUF: the PE warmup reads 256B/cycle and
            # must not share banks with the bxp planes the DVE writes early
            tc.tile_pool(name="warmp", bufs=1, side="right") as warmp,
            tc.tile_pool(name="psum", bufs=1, space="PSUM") as psum,
        ):
            # PE warmup fillers during the initial DMA-wait window. K=1
            # matmuls keep the PE busy a full 448 cycles per instr while
            # reading ~1B/cycle from a single partition, so they steal no
            # SBUF bandwidth from the concurrent head DMAs/signs/copies.
            # The memset goes on GpSimd, whose queue drains first at startup,
            # so the warmup starts ~1.5us earlier than a DVE memset allows.
            warm = warmp.tile([128, 512], dt.float8e4, tag="warm", name="warm")
            nc.gpsimd.memset(warm[0:1, :], 0.0)
            # dependency-free dummy activation: forces the ACT_TABLE_LOAD to
            # run at queue start instead of serializing behind the first
            # Sign's DMA-wait (reads one uninitialized byte, result unused)
            scr = warmp.tile([1, 2], dt.float8e4, tag="scr", name="scr")
            nc.scalar.activation(out=scr[0:1, 1:2], in_=scr[0:1, 0:1], func=AF.Sign)
            ps_warm = psum.tile([128, 512], dt.float32, tag="ps1", bufs=2,
                                name="ps_warm")
            N_WARM = 13
            for k in range(N_WARM):
                nc.tensor.matmul(
                    ps_warm[:, 0:448],
                    lhsT=warm[0:1, 0:128],
                    rhs=warm[0:1, 0:448],
                    start=(k == 0), stop=(k == N_WARM - 1),
                )

            # image-0 input DMA in row bands on the SP hardware-DGE queue,
            # band-major with the two c_blk halves adjacent, so the first
            # Sign (which reads both halves) unblocks after just two
            # transfers. The gpsimd software-DGE ring moves the weights in
            # parallel but is too slow to carry input bands (its data rate
            # serializes behind w1), and the Activation queue is kept free
            # of DMA triggers: its ~1.5us ACT_TABLE_LOAD preamble would push
            # the first Sign late.
            x_tiles = {}
            x0 = xin.tile([128, A, HW], dt.float32, tag="x_t", name="x_0")
            for lo, hi in BANDS_I0:
                for a in range(A):
                    nc.sync.dma_start(out=x0[:, a, lo * W:hi * W],
                                      in_=x_d[0, a, :, lo * W:hi * W])
            x_tiles[0] = x0

            ws = {}

            def wtile(conv, b, w_d):
                t = consts.tile([128, 9, A, 128], dt.float8e4, tag=f"w{conv}_{b}",
                                name=f"w{conv}_{b}")
                # one whole-tile DMA: software-DGE cost is per packet (128
                # row-packets either way), so splitting only adds overhead
                nc.gpsimd.dma_start(out=t[:], in_=w_d[b])
                ws[(conv, b)] = t

            # gpsimd trigger order tracks first-use time: w1 feeds conv1
            # from ~12us, cn feeds the first conv1 evacuation, w2 is not
            # consumed until ~25us in
            wtile(0, 0, w1_d)
            wtile(0, 1, w1_d)
            cns = []
            for b in range(A):
                t = consts.tile([128, 4], dt.float32, tag=f"cn_{b}", name=f"cn_{b}")
                nc.gpsimd.dma_start(out=t[:], in_=cn_d[b])
                cns.append(t)
            wtile(1, 0, w2_d)
            wtile(1, 1, w2_d)

            # sign planes [128, kj, c_blk, 58 rows, 56 cols] fp8, borders 0,
            # ping-ponged across images. plane[kj][.., rr, j] = xpad[.., rr, j+kj]
            bxp = [planes.tile([128, 3, A, ROWS, W], dt.float8e4, tag=f"bxp{j}",
                               name=f"bxp{j}") for j in range(2)]
            s2p = [planes.tile([128, 3, A, ROWS, W], dt.float8e4, tag=f"s2p{j}",
                               name=f"s2p{j}") for j in range(2)]

            def border_init(t, eng):
                # border-only init: zero the padding columns never
                # overwritten per image (kj0 col 0, kj2 col W-1). Pad rows
                # 0/57 are never read: the ki=0/2 taps that would touch them
                # are trimmed off the top/bottom chunks instead.
                eng.memset(t[:, 0, :, :, 0:1], 0.0)
                eng.memset(t[:, 2, :, :, W - 1:W], 0.0)

            # only bxp[0] is needed before the first matmul (DVE, ahead of
            # image 0's shifted kj-plane copies); the other three tiles init
            # on the otherwise-idle GpSimd engine so the DVE queue stays free
            border_init(bxp[0], nc.vector)
            for t in (bxp[1], s2p[0], s2p[1]):
                border_init(t, nc.gpsimd)

            BANK = 512

            def conv_group(src, conv, b, group, ps):
                # weights are stored tap-major in TAPS (consumption) order.
                # ki=0 taps at the top chunk (and ki=2 at the bottom) would
                # only read the zero pad row for their first (last) 56
                # columns, so those columns are trimmed; TAPS[0] has ki=1 and
                # stays full-width, so start=True always clears the whole
                # psum region.
                flat = src.rearrange("p kj a r c -> p kj a (r c)")
                for n_, t_ in enumerate(TAPS):
                    ki, kj = divmod(t_, 3)
                    for gi, ch in enumerate(group):
                        r0 = ch * RCH
                        lo = W if (ki == 0 and ch == 0) else 0
                        hi = W if (ki == 2 and ch == NCHUNK - 1) else 0
                        s0 = (r0 + ki) * W
                        nc.tensor.matmul(
                            ps[:, gi * BANK + lo:gi * BANK + CHUNK - hi],
                            lhsT=ws[(conv, b)][:, n_, :, :],
                            rhs=flat[:, kj, :, s0 + lo:s0 + CHUNK - hi],
                            start=(n_ == 0), stop=(n_ == 8),
                            perf_mode=DR,
                        )

            def psum_tile(group, nm):
                # chunks live at bank-aligned offsets; tail 64 fp32/bank unused
                return psum.tile([128, len(group) * BANK], dt.float32,
                                 tag=f"ps{len(group)}", bufs=3 if len(group) > 1 else 2,
                                 name=nm)

            def psum_chunks(ps, group):
                # [128, G, 448] view of the used part of each bank
                return ps.rearrange("p (g x) -> p g x", x=BANK)[:, :, 0:CHUNK]

            for i in range(n_img):
                j = i % 2
                if i in x_tiles:
                    x_t = x_tiles[i]
                else:
                    x_t = xin.tile([128, A, HW], dt.float32, tag="x_t", name=f"x_{i}")
                    nc.sync.dma_start(out=x_t[:],
                                      in_=x_d[i].rearrange("a k s -> k a s"))

                # binarize input: kj=1 and kj=0 planes on ScalarE, kj=2 via DVE
                # copy; image 0 banded to track its banded DMA
                xv = x_t.rearrange("p a (r c) -> p a r c", c=W)
                bands = BANDS_I0 if i == 0 else ((0, H),)
                if i == 0:
                    # head: ScalarE does only the kj=1 signs (both c_blk
                    # halves in one instruction — the Sign is on the first-MM
                    # critical path); kj=0/2 planes come from DVE shifted
                    # copies. kj=0 copies go first: the tap order consumes
                    # kj=0 three taps before kj=2
                    for lo, hi in bands:
                        nc.scalar.activation(
                            out=bxp[j][:, 1, :, 1 + lo:1 + hi, :],
                            in_=xv[:, :, lo:hi, :],
                            func=AF.Sign,
                        )
                        nc.vector.tensor_copy(
                            out=bxp[j][:, 0, :, 1 + lo:1 + hi, 1:W],
                            in_=bxp[j][:, 1, :, 1 + lo:1 + hi, 0:W - 1])
                        nc.vector.tensor_copy(
                            out=bxp[j][:, 2, :, 1 + lo:1 + hi, 0:W - 1],
                            in_=bxp[j][:, 1, :, 1 + lo:1 + hi, 1:W])
                else:
                    # NOTE: do NOT high_priority this block — the scheduler's
                    # DMA model underestimates the ~35us x-image load, and
                    # hoisting the signs ahead of the previous image's evacs
                    # in the in-order ScalarE queue head-of-line blocks them
                    # on x data (measured 11-17us PE stalls per image)
                    for lo, hi in bands:
                        for a in range(A):
                            nc.scalar.activation(
                                out=bxp[j][:, 1, a, 1 + lo:1 + hi, :],
                                in_=xv[:, a, lo:hi, :],
                                func=AF.Sign,
                            )
                        for a in range(A):
                            nc.scalar.activation(
                                out=bxp[j][:, 0, a, 1 + lo:1 + hi, 1:W],
                                in_=xv[:, a, lo:hi, 0:W - 1],
                                func=AF.Sign,
                            )
                        nc.vector.tensor_copy(
                            out=bxp[j][:, 2, :, 1 + lo:1 + hi, 0:W - 1],
                            in_=bxp[j][:, 1, :, 1 + lo:1 + hi, 1:W])

                # conv1 -> fused bn1+sign -> s2p (x3 shifted)
                conv1_iter = ([(b_, GROUPS_I0[g_]) for b_, g_ in CONV1_I0_ORDER]
                              if i == 0 else
                              [(b_, g_) for b_ in range(A) for g_ in GROUPS])
                for b, group in conv1_iter:
                    if True:
                        gr = len(group) * RCH
                        r0 = group[0] * RCH
                        ps = psum_tile(group, f"ps1_{i}_{b}_{group[0]}")
                        conv_group(bxp[j], 0, b, group, ps)
                        nc.scalar.activation(
                            out=s2p[j][:, 1, b, 1 + r0:1 + r0 + gr, :],
                            in_=psum_chunks(ps, group).rearrange(
                                "p g (r c) -> p g r c", c=W),
                            func=AF.Sign,
                            bias=cns[b][:, 1:2],
                            scale=cns[b][:, 0:1],
                        )
                        nc.vector.tensor_copy(
                            out=s2p[j][:, 0, b, 1 + r0:1 + r0 + gr, 1:W],
                            in_=s2p[j][:, 1, b, 1 + r0:1 + r0 + gr, 0:W - 1])
                        nc.vector.tensor_copy(
                            out=s2p[j][:, 2, b, 1 + r0:1 + r0 + gr, 0:W - 1],
                            in_=s2p[j][:, 1, b, 1 + r0:1 + r0 + gr, 1:W])

                out_t = outp.tile([128, A, HW], dt.float32, tag="out_t", name=f"out_{i}")

                # conv2 -> DVE (psum*a2)+x -> ScalarE relu(. + c2);
                # each b half DMAs out as soon as it completes. The last
                # image runs group-major so its out-DMA stream spreads over
                # the whole conv instead of bunching behind the final MMs.
                if i == n_img - 1:
                    conv2_iter = [(b_, g_) for g_ in GROUPS for b_ in range(A)]
                else:
                    conv2_iter = [(b_, g_) for b_ in range(A) for g_ in GROUPS]
                for bg, (b, group) in enumerate(conv2_iter):
                    if True:
                        gn = len(group) * CHUNK
                        s0 = group[0] * CHUNK
                        ps = psum_tile(group, f"ps2_{i}_{b}_{group[0]}")
                        conv_group(s2p[j], 1, b, group, ps)
                        rr = evac.tile([128, 2 * CHUNK], dt.float32, tag="rr",
                                       name=f"rr_{i}_{b}_{group[0]}")
                        last_img = i == n_img - 1
                        if last_img and b == A - 1 and group == GROUPS[-1]:
                            # final chunk: pipeline the evac in two halves and
                            # land each half on its own HWDGE ring, so the
                            # post-last-matmul chain is half length
                            for h, eng in ((0, nc.sync), (1, nc.scalar)):
                                c0, c1 = h * (gn // 2), (h + 1) * (gn // 2)
                                nc.vector.scalar_tensor_tensor(
                                    out=rr[:, c0:c1],
                                    in0=psum_chunks(ps, group)[:, :, c0:c1],
                                    scalar=cns[b][:, 2:3],
                                    in1=x_t[:, b, s0 + c0:s0 + c1],
                                    op0=OP.mult, op1=OP.add)
                                nc.scalar.activation(
                                    out=out_t[:, b, s0 + c0:s0 + c1],
                                    in_=rr[:, c0:c1],
                                    func=AF.Relu,
                                    bias=cns[b][:, 3:4],
                                )
                                eng.dma_start(
                                    out=out_d[i, b, :, s0 + c0:s0 + c1],
                                    in_=out_t[:, b, s0 + c0:s0 + c1])
                            continue
                        nc.vector.scalar_tensor_tensor(
                            out=rr[:, :gn], in0=psum_chunks(ps, group),
                            scalar=cns[b][:, 2:3],
                            in1=x_t[:, b, s0:s0 + gn],
                            op0=OP.mult, op1=OP.add)
                        nc.scalar.activation(
                            out=out_t[:, b, s0:s0 + gn],
                            in_=rr[:, :gn],
                            func=AF.Relu,
                            bias=cns[b][:, 3:4],
                        )
                        if last_img:
                            # tail: stream each finished group, alternating
                            # rings so no chunk queues behind the previous one
                            eng = nc.sync if bg % 2 == 0 else nc.scalar
                            eng.dma_start(out=out_d[i, b, :, s0:s0 + gn],
                                          in_=out_t[:, b, s0:s0 + gn])

                if i != n_img - 1:
                    nc.scalar.dma_start(out=out_d[i].rearrange("a k s -> k a s"),
                                        in_=out_t[:])

    nc.compile()
    return nc


def _get_program(n_img):
    if n_img not in _CACHE:
        _CACHE[n_img] = _build_program(n_img)
    return _CACHE[n_img]


def _prep_consts(w1, gamma1, beta1, mean1, var1, w2, gamma2, beta2, mean2, var2):
    import ml_dtypes

    def wprep(w):
        # [O, C, 3, 3] -> [co_blk b, ci k, tap t, ci_blk i, co m], sign in
        # fp8e4, with the tap axis permuted into TAPS (consumption) order
        s = np.sign(w.astype(np.float32)).reshape(A, 128, A, 128, 9)  # [b, m, i, k, t]
        t = np.ascontiguousarray(s.transpose(0, 3, 4, 2, 1))  # [b, k, t, i, m]
        return np.ascontiguousarray(t[:, :, TAPS]).astype(ml_dtypes.float8_e4m3)

    def bnfold(w, gamma, beta, mean, var):
        alpha = np.mean(np.abs(w.astype(np.float32)), axis=(1, 2, 3), dtype=np.float32)
        inv = (gamma.astype(np.float32)
               * (1.0 / np.sqrt(var.astype(np.float64) + EPS)).astype(np.float32))
        scale = alpha * inv
        bias = beta.astype(np.float32) - mean.astype(np.float32) * inv
        return scale, bias

    a1, c1 = bnfold(w1, gamma1, beta1, mean1, var1)
    a2, c2 = bnfold(w2, gamma2, beta2, mean2, var2)
    cn = np.ascontiguousarray(
        np.stack([a1, c1, a2, c2], axis=1).reshape(A, 128, 4)).astype(np.float32)
    return wprep(w1), wprep(w2), cn


def kernel(x, w1, gamma1, beta1, mean1, var1, w2, gamma2, beta2, mean2, var2):
    global LAST_RESULT
    from concourse.bass_utils import run_bass_kernel_spmd

    x, w1, gamma1, beta1, mean1, var1, w2, gamma2, beta2, mean2, var2 = (
        np.asarray(v) for v in
        (x, w1, gamma1, beta1, mean1, var1, w2, gamma2, beta2, mean2, var2))

    nc = _get_program(IMG_PER_CORE)
    w1t, w2t, cn = _prep_consts(w1, gamma1, beta1, mean1, var1,
                                w2, gamma2, beta2, mean2, var2)

    x = np.asarray(x, dtype=np.float32)
    xs = x.reshape(N_CORES, IMG_PER_CORE, A, 128, HW)
    in_maps = [
        {"x": xs[g], "w1t": w1t, "w2t": w2t, "cn": cn} for g in range(N_CORES)
    ]

    kwargs = {}
    if os.environ.get("BASS_KERNEL_TRACE"):
        _install_trace_shim()
        kwargs = dict(trace=True, tmpdir=os.environ.get("BASS_KERNEL_TRACE_DIR") or None)

    res = run_bass_kernel_spmd(nc, in_maps, list(range(N_CORES)), **kwargs)
    LAST_RESULT = res

    out = np.empty((N, C, H, W), dtype=np.float32)
    for g in range(N_CORES):
        out[g * IMG_PER_CORE:(g + 1) * IMG_PER_CORE] = (
            res.results[g]["out"].reshape(IMG_PER_CORE, C, H, W))
    return out


def _install_trace_shim():
    """This image lacks antenv.axon_hooks; recreate it so NTFF tracing works."""
    import sys, types
    if "antenv.axon_hooks" in sys.modules:
        return
    try:
        import antenv
        from trn_agent_boot.trn_boot import _ntff_profile_via_ctypes
    except ImportError:
        return
    mod = types.ModuleType("antenv.axon_hooks")
    _hook = [_ntff_profile_via_ctypes("/opt/axon/libaxon_pjrt.so")]
    mod.set_axon_ntff_profile_hook = lambda h: _hook.__setitem__(0, h)
    mod.get_axon_ntff_profile_hook = lambda: _hook[0]
    sys.modules["antenv.axon_hooks"] = mod
    antenv.axon_hooks = mod



# revision 32
# speedup vs baseline: 1.0136x; 1.0136x over previous
"""Trainium2 Bass kernel for an XNOR-Net BasicBlock (dense_cnn).

Computes, for x [64,256,56,56] (NCHW):
    h = xnor_conv3x3(x, w1) -> bn1 -> hardtanh -> xnor_conv3x3 -> bn2
    out = relu(h + x)

where xnor_conv binarizes activations with sign() and weights with
sign()*mean(|w|) (per output channel).

Strategy (v4, fp8 DoubleRow):
  - Data-parallel over batch: 8 images per NeuronCore x 8 cores.
  - Binarized activations (+-1) are exact in fp8e4; conv = 9 shifted
    matmuls per 3x3 tap with fp32 PSUM accumulation (exact integers).
  - perf_mode=DoubleRow contracts K=256 (both 128-channel blocks) per
    matmul: lhsT [128,2,128], rhs [128,2,448]. DoubleRow requires a 3D
    rhs AP with contiguous N, so sign planes are stored 3x, one copy per
    kj column shift, with row stride 56 (58 rows x 56 cols, borders 0).
    Window for tap (ki,kj), out-row-chunk r0 is then the contiguous run
    plane[kj][:, :, (r0+ki)*W : +N].
  - Chunks are processed in pairs sharing one 2-bank PSUM tile [128,896]
    (each matmul still targets a single bank), halving evacuation ops.
  - Epilogue fusions: conv1 evac = Sign(a1*psum + c1) on ScalarE writing
    the kj=1 plane (DVE makes the kj=0/2 shifted copies); conv2 evac =
    DVE (psum*a2)+x then ScalarE Relu(. + c2). All per-channel constants
    (alpha, bn scale/bias) are folded on the host. hardtanh is a no-op
    for the final output because conv2 only consumes sign(h).

Layouts (per core):
  x DRAM     [8, 2, 128, 3136]   (img, c_blk, c_in_blk, h*w) fp32
  w DRAM     [2, 128, 9, 2, 128] (co_blk, ci, tap, ci_blk, co) fp8 sign
  cn DRAM    [2, 128, 4]         (co_blk, co, {a1,c1,a2,c2}) fp32
  out DRAM   [8, 2, 128, 3136]   (img, co_blk, co, h*w) fp32
"""

import os
import numpy as np

N, C, H, W = 64, 256, 56, 56
EPS = 1e-5
N_CORES = 8
IMG_PER_CORE = N // N_CORES
A = 2                     # channel blocks of 128
ROWS = H + 2              # padded rows in a plane
PLANE = ROWS * W          # 3248 (multiple of 16 for DoubleRow dim1 step)
RCH = 8                   # output rows per PSUM chunk
CHUNK = RCH * W           # 448 fp32 <= 512 (one PSUM bank)
HW = H * W
NCHUNK = HW // CHUNK      # 7
GROUPS = [(0, 1), (2, 3), (4, 5), (6,)]   # chunk pairs -> one PSUM tile
GROUPS_I0 = [(0,), (1, 2), (3, 4), (5, 6)]  # img 0: tiny first group (fast start)
# img 0 conv1 (b, group-idx) order: interleave the two output halves so PE
# consumption tracks the banded x/w DMA arrival order; both b's of each
# row-group run before the next group, maximizing slack for later bands
# (group 0 needs only band 0, which lands ~2us before band 2)
CONV1_I0_ORDER = [(0, 0), (1, 0), (0, 1), (1, 1), (0, 2), (1, 2), (0, 3), (1, 3)]
BANDS_I0 = ((0, 9), (9, 17), (17, 25), (25, 41), (41, 56))  # x row-bands
# kj=1 taps first (plane-prep overlap); ki=1 first within each kj group so
# the start=True matmul is always full-width (ki=0/2 taps are trimmed at the
# top/bottom output-row chunks where they'd only read the zero pad row)
TAPS = [4, 1, 7, 3, 0, 6, 5, 2, 8]

_CACHE = {}
LAST_RESULT = None


def _build_program(n_img):
    import concourse.bacc as bacc
    import concourse.mybir as mybir
    import concourse.tile as tile

    dt = mybir.dt
    AF = mybir.ActivationFunctionType
    OP = mybir.AluOpType
    DR = mybir.MatmulPerfMode.DoubleRow

    nc = bacc.Bacc("TRN2", target_bir_lowering=False, debug=False)

    x_d = nc.dram_tensor("x", [n_img, A, 128, HW], dt.float32, kind="ExternalInput")
    w1_d = nc.dram_tensor("w1t", [A, 128, 9, A, 128], dt.float8e4, kind="ExternalInput")
    w2_d = nc.dram_tensor("w2t", [A, 128, 9, A, 128], dt.float8e4, kind="ExternalInput")
    cn_d = nc.dram_tensor("cn", [A, 128, 4], dt.float32, kind="ExternalInput")
    out_d = nc.dram_tensor("out", [n_img, A, 128, HW], dt.float32, kind="ExternalOutput")

    with tile.TileContext(nc) as tc:
        with (
            tc.tile_pool(name="consts", bufs=1) as consts,
            tc.tile_pool(name="planes", bufs=1) as planes,
            tc.tile_pool(name="xin", bufs=2) as xin,
            tc.tile_pool(name="outp", bufs=1) as outp,
            tc.tile_pool(name="evac", bufs=3) as evac,
            # far (right) side of SBUF: the PE warmup reads 256B/cycle and
            # must not share banks with the bxp planes the DVE writes early
            tc.tile_pool(name="warmp", bufs=1, side="right") as warmp,
            tc.tile_pool(name="psum", bufs=1, space="PSUM") as psum,
        ):
            # PE warmup fillers during the initial DMA-wait window. K=1
            # matmuls keep the PE busy a full 448 cycles per instr while
            # reading ~1B/cycle from a single partition, so they steal no
            # SBUF bandwidth from the concurrent head DMAs/signs/copies.
            # The memset goes on GpSimd, whose queue drains first at startup,
            # so the warmup starts ~1.5us earlier than a DVE memset allows.
            warm = warmp.tile([128, 512], dt.float8e4, tag="warm", name="warm")
            nc.gpsimd.memset(warm[0:1, :], 0.0)
            # dependency-free dummy activation: forces the ACT_TABLE_LOAD to
            # run at queue start instead of serializing behind the first
            # Sign's DMA-wait (reads one uninitialized byte, result unused)
            scr = warmp.tile([1, 2], dt.float8e4, tag="scr", name="scr")
            nc.scalar.activation(out=scr[0:1, 1:2], in_=scr[0:1, 0:1], func=AF.Sign)
            ps_warm = psum.tile([128, 512], dt.float32, tag="ps1", bufs=2,
                                name="ps_warm")
            N_WARM = 10
            for k in range(N_WARM):
                nc.tensor.matmul(
                    ps_warm[:, 0:448],
                    lhsT=warm[0:1, 0:128],
                    rhs=warm[0:1, 0:448],
                    start=(k == 0), stop=(k == N_WARM - 1),
                )

            # image-0 input DMA in row bands, the a=0 halves on the SP
            # hardware-DGE queue and the a=1 halves on the Activation one
            # (safe now that the dummy activation above preloads the ACT
            # table), so the two halves of each band transfer on parallel
            # queue streams and band 0 lands ~2us sooner than single-queue.
            # The gpsimd software-DGE ring moves the weights in parallel but
            # is too slow to carry input bands (its data rate serializes
            # behind w1).
            x_tiles = {}
            x0 = xin.tile([128, A, HW], dt.float32, tag="x_t", name="x_0")
            for lo, hi in BANDS_I0:
                for a in range(A):
                    eng = nc.sync if a == 0 else nc.scalar
                    eng.dma_start(out=x0[:, a, lo * W:hi * W],
                                  in_=x_d[0, a, :, lo * W:hi * W])
            x_tiles[0] = x0

            ws = {}

            def wtile(conv, b, w_d):
                t = consts.tile([128, 9, A, 128], dt.float8e4, tag=f"w{conv}_{b}",
                                name=f"w{conv}_{b}")
                # one whole-tile DMA: software-DGE cost is per packet (128
                # row-packets either way), so splitting only adds overhead
                nc.gpsimd.dma_start(out=t[:], in_=w_d[b])
                ws[(conv, b)] = t

            # gpsimd trigger order tracks first-use time: w1 feeds conv1
            # from ~12us, cn feeds the first conv1 evacuation, w2 is not
            # consumed until ~25us in
            wtile(0, 0, w1_d)
            wtile(0, 1, w1_d)
            cns = []
            for b in range(A):
                t = consts.tile([128, 4], dt.float32, tag=f"cn_{b}", name=f"cn_{b}")
                nc.gpsimd.dma_start(out=t[:], in_=cn_d[b])
                cns.append(t)
            wtile(1, 0, w2_d)
            wtile(1, 1, w2_d)

            # sign planes [128, kj, c_blk, 58 rows, 56 cols] fp8, borders 0,
            # ping-ponged across images. plane[kj][.., rr, j] = xpad[.., rr, j+kj]
            bxp = [planes.tile([128, 3, A, ROWS, W], dt.float8e4, tag=f"bxp{j}",
                               name=f"bxp{j}") for j in range(2)]
            s2p = [planes.tile([128, 3, A, ROWS, W], dt.float8e4, tag=f"s2p{j}",
                               name=f"s2p{j}") for j in range(2)]

            def border_init(t, eng):
                # border-only init: zero the padding columns never
                # overwritten per image (kj0 col 0, kj2 col W-1). Pad rows
                # 0/57 are never read: the ki=0/2 taps that would touch them
                # are trimmed off the top/bottom chunks instead.
                eng.memset(t[:, 0, :, :, 0:1], 0.0)
                eng.memset(t[:, 2, :, :, W - 1:W], 0.0)

            # only bxp[0] is needed before the first matmul (DVE, ahead of
            # image 0's shifted kj-plane copies); the other three tiles init
            # on the otherwise-idle GpSimd engine so the DVE queue stays free
            border_init(bxp[0], nc.vector)
            for t in (bxp[1], s2p[0], s2p[1]):
                border_init(t, nc.gpsimd)

            BANK = 512

            def conv_group(src, conv, b, group, ps, span=None):
                # weights are stored tap-major in TAPS (consumption) order.
                # ki=0 taps at the top chunk (and ki=2 at the bottom) would
                # only read the zero pad row for their first (last) 56
                # columns, so those columns are trimmed; TAPS[0] has ki=1 and
                # stays full-width, so start=True always clears the whole
                # psum region. span=(c0,c1) restricts a single-chunk group to
                # a column sub-range (used to split the final chunk so its
                # first half evacuates while the second half's matmuls run).
                flat = src.rearrange("p kj a r c -> p kj a (r c)")
                for n_, t_ in enumerate(TAPS):
                    ki, kj = divmod(t_, 3)
                    for gi, ch in enumerate(group):
                        r0 = ch * RCH
                        lo = W if (ki == 0 and ch == 0) else 0
                        hi = W if (ki == 2 and ch == NCHUNK - 1) else 0
                        c0, c1 = (0, CHUNK) if span is None else span
                        lo, hi = max(lo, c0), min(CHUNK - hi, c1)
                        s0 = (r0 + ki) * W
                        nc.tensor.matmul(
                            ps[:, gi * BANK + lo - c0:gi * BANK + hi - c0],
                            lhsT=ws[(conv, b)][:, n_, :, :],
                            rhs=flat[:, kj, :, s0 + lo:s0 + hi],
                            start=(n_ == 0), stop=(n_ == 8),
                            perf_mode=DR,
                        )

            def psum_tile(group, nm):
                # chunks live at bank-aligned offsets; tail 64 fp32/bank unused
                return psum.tile([128, len(group) * BANK], dt.float32,
                                 tag=f"ps{len(group)}", bufs=3 if len(group) > 1 else 2,
                                 name=nm)

            def psum_chunks(ps, group):
                # [128, G, 448] view of the used part of each bank
                return ps.rearrange("p (g x) -> p g x", x=BANK)[:, :, 0:CHUNK]

            for i in range(n_img):
                j = i % 2
                if i in x_tiles:
                    x_t = x_tiles[i]
                else:
                    x_t = xin.tile([128, A, HW], dt.float32, tag="x_t", name=f"x_{i}")
                    nc.sync.dma_start(out=x_t[:],
                                      in_=x_d[i].rearrange("a k s -> k a s"))

                # binarize input: kj=1 and kj=0 planes on ScalarE, kj=2 via DVE
                # copy; image 0 banded to track its banded DMA
                xv = x_t.rearrange("p a (r c) -> p a r c", c=W)
                # NOTE: banding image 1's signs into row-halves was tried to
                # shrink the ~2.5us image-1 boundary stall; the scheduler
                # packed the smaller ops even later and tripled the gap
                bands = BANDS_I0 if i == 0 else ((0, H),)
                if i == 0:
                    # head: ScalarE does only the kj=1 signs (both c_blk
                    # halves in one instruction — the Sign is on the first-MM
                    # critical path); kj=0/2 planes come from DVE shifted
                    # copies. kj=0 copies go first: the tap order consumes
                    # kj=0 three taps before kj=2
                    for lo, hi in bands:
                        nc.scalar.activation(
                            out=bxp[j][:, 1, :, 1 + lo:1 + hi, :],
                            in_=xv[:, :, lo:hi, :],
                            func=AF.Sign,
                        )
                        nc.vector.tensor_copy(
                            out=bxp[j][:, 0, :, 1 + lo:1 + hi, 1:W],
                            in_=bxp[j][:, 1, :, 1 + lo:1 + hi, 0:W - 1])
                        nc.vector.tensor_copy(
                            out=bxp[j][:, 2, :, 1 + lo:1 + hi, 0:W - 1],
                            in_=bxp[j][:, 1, :, 1 + lo:1 + hi, 1:W])
                else:
                    # NOTE: do NOT high_priority this block — the scheduler's
                    # DMA model underestimates the ~35us x-image load, and
                    # hoisting the signs ahead of the previous image's evacs
                    # in the in-order ScalarE queue head-of-line blocks them
                    # on x data (measured 11-17us PE stalls per image)
                    for lo, hi in bands:
                        for a in range(A):
                            nc.scalar.activation(
                                out=bxp[j][:, 1, a, 1 + lo:1 + hi, :],
                                in_=xv[:, a, lo:hi, :],
                                func=AF.Sign,
                            )
                        for a in range(A):
                            nc.scalar.activation(
                                out=bxp[j][:, 0, a, 1 + lo:1 + hi, 1:W],
                                in_=xv[:, a, lo:hi, 0:W - 1],
                                func=AF.Sign,
                            )
                        nc.vector.tensor_copy(
                            out=bxp[j][:, 2, :, 1 + lo:1 + hi, 0:W - 1],
                            in_=bxp[j][:, 1, :, 1 + lo:1 + hi, 1:W])

                # conv1 -> fused bn1+sign -> s2p (x3 shifted)
                conv1_iter = ([(b_, GROUPS_I0[g_]) for b_, g_ in CONV1_I0_ORDER]
                              if i == 0 else
                              [(b_, g_) for b_ in range(A) for g_ in GROUPS])
                for b, group in conv1_iter:
                    if True:
                        gr = len(group) * RCH
                        r0 = group[0] * RCH
                        ps = psum_tile(group, f"ps1_{i}_{b}_{group[0]}")
                        conv_group(bxp[j], 0, b, group, ps)
                        nc.scalar.activation(
                            out=s2p[j][:, 1, b, 1 + r0:1 + r0 + gr, :],
                            in_=psum_chunks(ps, group).rearrange(
                                "p g (r c) -> p g r c", c=W),
                            func=AF.Sign,
                            bias=cns[b][:, 1:2],
                            scale=cns[b][:, 0:1],
                        )
                        nc.vector.tensor_copy(
                            out=s2p[j][:, 0, b, 1 + r0:1 + r0 + gr, 1:W],
                            in_=s2p[j][:, 1, b, 1 + r0:1 + r0 + gr, 0:W - 1])
                        nc.vector.tensor_copy(
                            out=s2p[j][:, 2, b, 1 + r0:1 + r0 + gr, 0:W - 1],
                            in_=s2p[j][:, 1, b, 1 + r0:1 + r0 + gr, 1:W])

                out_t = outp.tile([128, A, HW], dt.float32, tag="out_t", name=f"out_{i}")

                # conv2 -> DVE (psum*a2)+x -> ScalarE relu(. + c2);
                # each b half DMAs out as soon as it completes. The last
                # image runs group-major so its out-DMA stream spreads over
                # the whole conv instead of bunching behind the final MMs.
                if i == n_img - 1:
                    conv2_iter = [(b_, g_) for g_ in GROUPS for b_ in range(A)]
                else:
                    conv2_iter = [(b_, g_) for b_ in range(A) for g_ in GROUPS]
                for bg, (b, group) in enumerate(conv2_iter):
                    if True:
                        gn = len(group) * CHUNK
                        s0 = group[0] * CHUNK
                        rr = evac.tile([128, 2 * CHUNK], dt.float32, tag="rr",
                                       name=f"rr_{i}_{b}_{group[0]}")
                        last_img = i == n_img - 1
                        if last_img and b == A - 1 and group == GROUPS[-1]:
                            # final chunk: two half-width accumulation groups
                            # so the first half's evac + store overlap the
                            # second half's matmuls, and each half lands on
                            # its own HWDGE ring — the post-last-matmul chain
                            # covers only 224 columns
                            for h, eng in ((0, nc.sync), (1, nc.scalar)):
                                c0, c1 = h * (gn // 2), (h + 1) * (gn // 2)
                                ps_h = psum.tile([128, BANK], dt.float32,
                                                 tag="ps1", bufs=2,
                                                 name=f"ps2f_{h}")
                                conv_group(s2p[j], 1, b, group, ps_h,
                                           span=(c0, c1))
                                nc.vector.scalar_tensor_tensor(
                                    out=rr[:, c0:c1],
                                    in0=ps_h[:, 0:c1 - c0],
                                    scalar=cns[b][:, 2:3],
                                    in1=x_t[:, b, s0 + c0:s0 + c1],
                                    op0=OP.mult, op1=OP.add)
                                nc.scalar.activation(
                                    out=out_t[:, b, s0 + c0:s0 + c1],
                                    in_=rr[:, c0:c1],
                                    func=AF.Relu,
                                    bias=cns[b][:, 3:4],
                                )
                                eng.dma_start(
                                    out=out_d[i, b, :, s0 + c0:s0 + c1],
                                    in_=out_t[:, b, s0 + c0:s0 + c1])
                            continue
                        ps = psum_tile(group, f"ps2_{i}_{b}_{group[0]}")
                        conv_group(s2p[j], 1, b, group, ps)
                        nc.vector.scalar_tensor_tensor(
                            out=rr[:, :gn], in0=psum_chunks(ps, group),
                            scalar=cns[b][:, 2:3],
                            in1=x_t[:, b, s0:s0 + gn],
                            op0=OP.mult, op1=OP.add)
                        nc.scalar.activation(
                            out=out_t[:, b, s0:s0 + gn],
                            in_=rr[:, :gn],
                            func=AF.Relu,
                            bias=cns[b][:, 3:4],
                        )
                        if last_img:
                            # tail: stream each finished group, alternating
                            # rings so no chunk queues behind the previous one
                            eng = nc.sync if bg % 2 == 0 else nc.scalar
                            eng.dma_start(out=out_d[i, b, :, s0:s0 + gn],
                                          in_=out_t[:, b, s0:s0 + gn])

                if i != n_img - 1:
                    nc.scalar.dma_start(out=out_d[i].rearrange("a k s -> k a s"),
                                        in_=out_t[:])

    nc.compile()
    return nc


def _get_program(n_img):
    if n_img not in _CACHE:
        _CACHE[n_img] = _build_program(n_img)
    return _CACHE[n_img]


def _prep_consts(w1, gamma1, beta1, mean1, var1, w2, gamma2, beta2, mean2, var2):
    import ml_dtypes

    def wprep(w):
        # [O, C, 3, 3] -> [co_blk b, ci k, tap t, ci_blk i, co m], sign in
        # fp8e4, with the tap axis permuted into TAPS (consumption) order
        s = np.sign(w.astype(np.float32)).reshape(A, 128, A, 128, 9)  # [b, m, i, k, t]
        t = np.ascontiguousarray(s.transpose(0, 3, 4, 2, 1))  # [b, k, t, i, m]
        return np.ascontiguousarray(t[:, :, TAPS]).astype(ml_dtypes.float8_e4m3)

    def bnfold(w, gamma, beta, mean, var):
        alpha = np.mean(np.abs(w.astype(np.float32)), axis=(1, 2, 3), dtype=np.float32)
        inv = (gamma.astype(np.float32)
               * (1.0 / np.sqrt(var.astype(np.float64) + EPS)).astype(np.float32))
        scale = alpha * inv
        bias = beta.astype(np.float32) - mean.astype(np.float32) * inv
        return scale, bias

    a1, c1 = bnfold(w1, gamma1, beta1, mean1, var1)
    a2, c2 = bnfold(w2, gamma2, beta2, mean2, var2)
    cn = np.ascontiguousarray(
        np.stack([a1, c1, a2, c2], axis=1).reshape(A, 128, 4)).astype(np.float32)
    return wprep(w1), wprep(w2), cn


def kernel(x, w1, gamma1, beta1, mean1, var1, w2, gamma2, beta2, mean2, var2):
    global LAST_RESULT
    from concourse.bass_utils import run_bass_kernel_spmd

    x, w1, gamma1, beta1, mean1, var1, w2, gamma2, beta2, mean2, var2 = (
        np.asarray(v) for v in
        (x, w1, gamma1, beta1, mean1, var1, w2, gamma2, beta2, mean2, var2))

    nc = _get_program(IMG_PER_CORE)
    w1t, w2t, cn = _prep_consts(w1, gamma1, beta1, mean1, var1,
                                w2, gamma2, beta2, mean2, var2)

    x = np.asarray(x, dtype=np.float32)
    xs = x.reshape(N_CORES, IMG_PER_CORE, A, 128, HW)
    in_maps = [
        {"x": xs[g], "w1t": w1t, "w2t": w2t, "cn": cn} for g in range(N_CORES)
    ]

    kwargs = {}
    if os.environ.get("BASS_KERNEL_TRACE"):
        _install_trace_shim()
        kwargs = dict(trace=True, tmpdir=os.environ.get("BASS_KERNEL_TRACE_DIR") or None)

    res = run_bass_kernel_spmd(nc, in_maps, list(range(N_CORES)), **kwargs)
    LAST_RESULT = res

    out = np.empty((N, C, H, W), dtype=np.float32)
    for g in range(N_CORES):
        out[g * IMG_PER_CORE:(g + 1) * IMG_PER_CORE] = (
            res.results[g]["out"].reshape(IMG_PER_CORE, C, H, W))
    return out


def _install_trace_shim():
    """This image lacks antenv.axon_hooks; recreate it so NTFF tracing works."""
    import sys, types
    if "antenv.axon_hooks" in sys.modules:
        return
    try:
        import antenv
        from trn_agent_boot.trn_boot import _ntff_profile_via_ctypes
    except ImportError:
        return
    mod = types.ModuleType("antenv.axon_hooks")
    _hook = [_ntff_profile_via_ctypes("/opt/axon/libaxon_pjrt.so")]
    mod.set_axon_ntff_profile_hook = lambda h: _hook.__setitem__(0, h)
    mod.get_axon_ntff_profile_hook = lambda: _hook[0]
    sys.modules["antenv.axon_hooks"] = mod
    antenv.axon_hooks = mod

